# revision 13
# baseline (speedup 1.0000x reference)
"""Trainium2 Bass kernel for nn_Attn_14078902796904.

Computes attn = softmax(encoder_outputs @ hidden) for
encoder_outputs [65536, 1024] f32, hidden [1024] f32 -> [1, 1, 65536] f32.

Strategy (sequence-parallel across 8 NeuronCores):
  - Core c gets rows [c*8192, (c+1)*8192) of encoder_outputs; hidden is
    replicated (host pre-broadcasts it to [128, 4096] so every SBUF
    partition holds copies).
  - On-core: stream the 32 MB shard through SBUF in [128, nb*1024]
    chunks (DMA-paced, ~358 GB/s); Vector engine multiplies each chunk
    by hidden; per-1024-segment row reductions run mostly on the Scalar
    engine (activation accum_out, its own datapath) with a couple of
    mid-stream chunks on Vector -> energies [128, 64].  Per-partition
    max / subtract / Exp(+accum sums) give an unnormalized local
    softmax.  The last chunks are tapered smaller to shorten the
    post-DMA tail.
  - Host: flash-softmax recombination over the 8*128 partial (max, sum)
    pairs, scale + concat -> full output.
"""

import numpy as np

import concourse.bass as bass
import concourse.tile as tile
from concourse import bacc, mybir
from concourse.bass_utils import run_bass_kernel_spmd

S = 65536
H = 1024
N_CORES = 8
SC = S // N_CORES          # 8192 rows per core
P = 128                    # partitions
NT = SC // P               # 64 blocks of 128 rows per core
GMAX = 4                   # max blocks per DMA chunk (2 MB)

# chunk sizes in blocks; small priming chunks at the start (compute ramps
# up sooner) and tapered at the end (shorter post-DMA tail)
CHUNKS = [4] * 15 + [2, 2]
assert sum(CHUNKS) == NT

# chunk index -> which engine reduces it (multiplies all on Vector).
# Empty: a mid-stream Vector reduce stalls the Scalar engine's in-order
# energies writes behind it.
DVE_REDUCE = {0}

INP_BUFS = 6
PROD_BUFS = 4

_DT = mybir.dt.float32


def _build_nc():
    nc = bacc.Bacc("TRN2", target_bir_lowering=False, debug=False,
                   num_devices=N_CORES)
    enc = nc.dram_tensor("enc", [SC, H], _DT, kind="ExternalInput")
    hid = nc.dram_tensor("hid", [P, H], _DT, kind="ExternalInput")
    # out[:, 0:64] = probs, out[:, 64] = maxs, out[:, 65] = sums
    out = nc.dram_tensor("out", [P, NT + 2], _DT, kind="ExternalOutput")

    # enc_r[p, n, h] = enc[n*128 + p, h]
    enc_r = enc.ap().rearrange("(n p) h -> p n h", p=P)

    with tile.TileContext(nc) as tc:
        with (
            tc.tile_pool(name="inp", bufs=INP_BUFS) as inp_pool,
            tc.tile_pool(name="prod", bufs=PROD_BUFS) as prod_pool,
            tc.tile_pool(name="consts", bufs=1) as consts,
            tc.tile_pool(name="small", bufs=1) as small,
        ):
            hidrep = consts.tile([P, H], _DT)
            nc.sync.dma_start(hidrep[:], hid.ap())

            energies = small.tile([P, NT], _DT)

            blk = 0
            for g, nb in enumerate(CHUNKS):
                t_in = inp_pool.tile([P, GMAX * H], _DT, tag="t_in")
                nc.sync.dma_start(
                    t_in[:, :nb * H].rearrange("p (b h) -> p b h", h=H),
                    enc_r[:, blk:blk + nb, :],
                )
                prod = prod_pool.tile([P, GMAX * H], _DT, tag="prod")
                hid_bc = hidrep[:].rearrange(
                    "p (o h) -> p o h", o=1).broadcast_to((P, nb, H))
                nc.vector.tensor_mul(
                    prod[:, :nb * H].rearrange("p (b h) -> p b h", h=H),
                    t_in[:, :nb * H].rearrange("p (b h) -> p b h", h=H),
                    hid_bc,
                )

                if g in DVE_REDUCE:
                    nc.vector.reduce_sum(
                        energies[:, blk:blk + nb],
                        prod[:, :nb * H].rearrange("p (b h) -> p b h", h=H),
                        axis=mybir.AxisListType.X,
                    )
                else:
                    for j in range(nb):
                        seg = prod[:, j * H:(j + 1) * H]
                        nc.scalar.activation(
                            seg, seg,
                            mybir.ActivationFunctionType.Identity,
                            accum_out=energies[:, blk + j:blk + j + 1],
                        )
                blk += nb

            em = small.tile([P, 1], _DT)
            nc.vector.reduce_max(em[:], energies[:], axis=mybir.AxisListType.X)
            x = small.tile([P, NT], _DT)
            nc.vector.tensor_scalar_sub(x[:], energies[:], em[:])
            pt = small.tile([P, NT + 2], _DT)
            st = small.tile([P, 1], _DT)
            nc.scalar.activation(
                pt[:, :NT], x[:], mybir.ActivationFunctionType.Exp,
                accum_out=st[:],
            )
            nc.vector.tensor_copy(pt[:, NT:NT + 1], em[:])
            nc.vector.tensor_copy(pt[:, NT + 1:NT + 2], st[:])
            nc.sync.dma_start(out.ap(), pt[:])
    nc.compile()
    return nc


_NC_CACHE = None


def _get_nc():
    global _NC_CACHE
    if _NC_CACHE is None:
        _NC_CACHE = _build_nc()
    return _NC_CACHE


def run_device(hidden, encoder_outputs, **spmd_kwargs):
    """Run the per-core kernels; returns (list of per-core result dicts,
    BassKernelResults)."""
    hidden = np.asarray(hidden, dtype=np.float32)
    encoder_outputs = np.asarray(encoder_outputs, dtype=np.float32)
    hidrep = np.ascontiguousarray(np.broadcast_to(hidden, (P, H)))
    in_maps = [
        {
            "enc": np.ascontiguousarray(encoder_outputs[c * SC:(c + 1) * SC]),
            "hid": hidrep,
        }
        for c in range(N_CORES)
    ]
    res = run_bass_kernel_spmd(_get_nc(), in_maps, list(range(N_CORES)), **spmd_kwargs)
    return res.results, res


def combine(results):
    """Flash-softmax recombination of per-core partials -> [1, 1, S] f32."""
    outs = np.stack([r["out"] for r in results]).astype(np.float64)  # [8,128,66]
    probs = outs[:, :, :NT]                     # [8,128,64]
    maxs = outs[:, :, NT:NT + 1]                # [8,128,1]
    sums = outs[:, :, NT + 1:NT + 2]            # [8,128,1]
    M = maxs.max()
    scale = np.exp(maxs - M)                    # [8,128,1]
    Z = (sums * scale).sum()
    attn = probs * scale / Z                    # [8,128,64]
    # local row order: s_local = t*128 + p, so transpose [p, t] -> [t, p]
    attn = attn.transpose(0, 2, 1).reshape(S)
    return attn.astype(np.float32)[None, None, :]


def kernel(hidden, encoder_outputs):
    results, _ = run_device(hidden, encoder_outputs)
    return combine(results)


# revision 14
# speedup vs baseline: 1.1324x; 1.1324x over previous
"""Trainium2 Bass kernel for nn_Attn_14078902796904.

Computes attn = softmax(encoder_outputs @ hidden) for
encoder_outputs [65536, 1024] f32, hidden [1024] f32 -> [1, 1, 65536] f32.

Strategy (sequence-parallel across 8 NeuronCores):
  - Core c gets rows [c*8192, (c+1)*8192) of encoder_outputs; hidden is
    replicated (host pre-broadcasts it to [128, 4096] so every SBUF
    partition holds copies).
  - On-core: stream the 32 MB shard through SBUF in [128, nb*1024]
    chunks (DMA-paced, ~358 GB/s); Vector engine multiplies each chunk
    by hidden; per-1024-segment row reductions run mostly on the Scalar
    engine (activation accum_out, its own datapath) with a couple of
    mid-stream chunks on Vector -> energies [128, 64].  Per-partition
    max / subtract / Exp(+accum sums) give an unnormalized local
    softmax.  The last chunks are tapered smaller to shorten the
    post-DMA tail.
  - Host: flash-softmax recombination over the 8*128 partial (max, sum)
    pairs, scale + concat -> full output.
"""

import numpy as np

import concourse.bass as bass
import concourse.tile as tile
from concourse import bacc, mybir
from concourse.bass_utils import run_bass_kernel_spmd

S = 65536
H = 1024
N_CORES = 8
SC = S // N_CORES          # 8192 rows per core
P = 128                    # partitions
NT = SC // P               # 64 blocks of 128 rows per core
GMAX = 4                   # max blocks per DMA chunk (2 MB)

# chunk sizes in blocks; small priming chunks at the start (compute ramps
# up sooner) and tapered at the end (shorter post-DMA tail)
CHUNKS = [4] * 15 + [2, 2]
assert sum(CHUNKS) == NT

# chunk index -> which engine reduces it (multiplies all on Vector).
# Empty: a mid-stream Vector reduce stalls the Scalar engine's in-order
# energies writes behind it.
DVE_REDUCE = set()

INP_BUFS = 6
PROD_BUFS = 4

_DT = mybir.dt.float32


def _build_nc():
    nc = bacc.Bacc("TRN2", target_bir_lowering=False, debug=False,
                   num_devices=N_CORES)
    enc = nc.dram_tensor("enc", [SC, H], _DT, kind="ExternalInput")
    hid = nc.dram_tensor("hid", [P, H], _DT, kind="ExternalInput")
    # out[:, 0:64] = probs, out[:, 64] = maxs, out[:, 65] = sums
    out = nc.dram_tensor("out", [P, NT + 2], _DT, kind="ExternalOutput")

    # enc_r[p, n, h] = enc[n*128 + p, h]
    enc_r = enc.ap().rearrange("(n p) h -> p n h", p=P)

    with tile.TileContext(nc) as tc:
        with (
            tc.tile_pool(name="inp", bufs=INP_BUFS) as inp_pool,
            tc.tile_pool(name="prod", bufs=PROD_BUFS) as prod_pool,
            tc.tile_pool(name="consts", bufs=1) as consts,
            tc.tile_pool(name="small", bufs=1) as small,
        ):
            hidrep = consts.tile([P, H], _DT)
            nc.sync.dma_start(hidrep[:], hid.ap())

            energies = small.tile([P, NT], _DT)

            blk = 0
            for g, nb in enumerate(CHUNKS):
                t_in = inp_pool.tile([P, GMAX * H], _DT, tag="t_in")
                nc.sync.dma_start(
                    t_in[:, :nb * H].rearrange("p (b h) -> p b h", h=H),
                    enc_r[:, blk:blk + nb, :],
                )
                prod = prod_pool.tile([P, GMAX * H], _DT, tag="prod")
                hid_bc = hidrep[:].rearrange(
                    "p (o h) -> p o h", o=1).broadcast_to((P, nb, H))
                nc.vector.tensor_mul(
                    prod[:, :nb * H].rearrange("p (b h) -> p b h", h=H),
                    t_in[:, :nb * H].rearrange("p (b h) -> p b h", h=H),
                    hid_bc,
                )

                if g in DVE_REDUCE:
                    nc.vector.reduce_sum(
                        energies[:, blk:blk + nb],
                        prod[:, :nb * H].rearrange("p (b h) -> p b h", h=H),
                        axis=mybir.AxisListType.X,
                    )
                else:
                    for j in range(nb):
                        seg = prod[:, j * H:(j + 1) * H]
                        nc.scalar.activation(
                            seg, seg,
                            mybir.ActivationFunctionType.Identity,
                            accum_out=energies[:, blk + j:blk + j + 1],
                        )
                blk += nb

            em = small.tile([P, 1], _DT)
            nc.vector.reduce_max(em[:], energies[:], axis=mybir.AxisListType.X)
            x = small.tile([P, NT], _DT)
            nc.vector.tensor_scalar_sub(x[:], energies[:], em[:])
            pt = small.tile([P, NT + 2], _DT)
            st = small.tile([P, 1], _DT)
            nc.scalar.activation(
                pt[:, :NT], x[:], mybir.ActivationFunctionType.Exp,
                accum_out=st[:],
            )
            nc.vector.tensor_copy(pt[:, NT:NT + 1], em[:])
            nc.vector.tensor_copy(pt[:, NT + 1:NT + 2], st[:])
            nc.sync.dma_start(out.ap(), pt[:])
    nc.compile()
    return nc


_NC_CACHE = None


def _get_nc():
    global _NC_CACHE
    if _NC_CACHE is None:
        _NC_CACHE = _build_nc()
    return _NC_CACHE


def run_device(hidden, encoder_outputs, **spmd_kwargs):
    """Run the per-core kernels; returns (list of per-core result dicts,
    BassKernelResults)."""
    hidden = np.asarray(hidden, dtype=np.float32)
    encoder_outputs = np.asarray(encoder_outputs, dtype=np.float32)
    hidrep = np.ascontiguousarray(np.broadcast_to(hidden, (P, H)))
    in_maps = [
        {
            "enc": np.ascontiguousarray(encoder_outputs[c * SC:(c + 1) * SC]),
            "hid": hidrep,
        }
        for c in range(N_CORES)
    ]
    res = run_bass_kernel_spmd(_get_nc(), in_maps, list(range(N_CORES)), **spmd_kwargs)
    return res.results, res


def combine(results):
    """Flash-softmax recombination of per-core partials -> [1, 1, S] f32."""
    outs = np.stack([r["out"] for r in results]).astype(np.float64)  # [8,128,66]
    probs = outs[:, :, :NT]                     # [8,128,64]
    maxs = outs[:, :, NT:NT + 1]                # [8,128,1]
    sums = outs[:, :, NT + 1:NT + 2]            # [8,128,1]
    M = maxs.max()
    scale = np.exp(maxs - M)                    # [8,128,1]
    Z = (sums * scale).sum()
    attn = probs * scale / Z                    # [8,128,64]
    # local row order: s_local = t*128 + p, so transpose [p, t] -> [t, p]
    attn = attn.transpose(0, 2, 1).reshape(S)
    return attn.astype(np.float32)[None, None, :]


def kernel(hidden, encoder_outputs):
    results, _ = run_device(hidden, encoder_outputs)
    return combine(results)
